# revision 16
# baseline (speedup 1.0000x reference)
"""GINConv (sum-aggregation + 2-layer MLP) on 8 Trainium2 NeuronCores.

Strategy (see sharding hint): shard destination nodes across the 8 cores
(12500 rows each).  Edges are partitioned by destination core.  Inside a
core, edges are grouped by 256-wide destination "super-windows" and by
source quartile (so gather indices fit in int16).  The per-edge source
features are fetched with the HW dma_gather (SWDGE) primitive from a
replicated fp16 copy of x; the scatter-add is performed locally on the
tensor engine as  aggT[64f x 256d] += G_chunk[128e x 64f]^T @ onehot[128e x 256d]
with the one-hot built on the vector engine via tensor_scalar(is_equal)
against an iota row.  The MLP runs in the same transposed orientation
(features on partitions), so no transposes are needed on device; the host
transposes the final [64, 12544] output tiles back.

All 8 cores execute one identical NEFF (SPMD) - the chunk grid is padded
to the max across cores so the static schedule is core-independent.
"""

import numpy as np

D = 64          # feature dim
DP = 128        # padded feature dim (fp16 row = 256B, dma_gather elem size)
SWW = 256       # dsts per super-window (psum tile free dim)
NQ = 4          # source quartiles (gather idx must fit int16)
CHUNK = 128     # edges per matmul chunk (PE contraction dim)


def _plan_and_pack(x, edge_index, n_cores=8, swb=None):
    """Host-side: shard/sort/pad edges; build per-core packed index arrays.

    Returns (plan, per_core_inputs).
    """
    N = x.shape[0]
    E = edge_index.shape[1]
    npc = -(-N // n_cores)                  # nodes per core
    nsw = -(-npc // SWW)                    # super-windows per core
    npc_pad = nsw * SWW
    if swb is None:
        swb = 7 if nsw % 7 == 0 else 1      # super-windows per batch
    while nsw % swb != 0:
        swb -= 1
    nb = nsw // swb                         # batches
    qr = -(-N // NQ)                        # rows per source quartile
    assert qr <= 32767

    src = np.asarray(edge_index[0], dtype=np.int64)
    dst = np.asarray(edge_index[1], dtype=np.int64)

    # group sizes per (core, sw, q)
    core = dst // npc
    dl = dst - core * npc
    sw = dl // SWW
    q = src // qr
    gid = (core * nsw + sw) * NQ + q
    counts = np.bincount(gid, minlength=n_cores * nsw * NQ).reshape(
        n_cores, nsw, NQ
    )
    # uniform chunk grid: max over cores, at least 1 chunk
    cwq = np.maximum(1, -(-counts.max(axis=0) // CHUNK))      # [nsw, NQ]
    slots_g = cwq * CHUNK                                     # [nsw, NQ]

    # per (batch, q) stream sizes and offsets
    slots_bq = np.zeros((nb, NQ), np.int64)
    for b in range(nb):
        slots_bq[b] = slots_g[b * swb:(b + 1) * swb].sum(axis=0)
    tot = int(slots_bq.sum())
    totch = tot // CHUNK

    # chunk-base of group (sw, q) inside its (b, q) stream
    gbase = np.zeros((nsw, NQ), np.int64)
    for b in range(nb):
        acc = np.zeros(NQ, np.int64)
        for s in range(b * swb, (b + 1) * swb):
            gbase[s] = acc
            acc += cwq[s]
    # free-dim int16-col offset of (b, q) idx stream in the packed gidx array
    off16 = np.zeros((nb, NQ), np.int64)
    # chunk-column offset of (b, q) inside the packed dstloc array
    offch = np.zeros((nb, NQ), np.int64)
    a16 = 0
    ach = 0
    for b in range(nb):
        for qq in range(NQ):
            off16[b, qq] = a16
            offch[b, qq] = ach
            a16 += slots_bq[b, qq] // 16
            ach += slots_bq[b, qq] // CHUNK
    bco = np.zeros(nb + 1, np.int64)        # batch chunk-col starts
    for b in range(nb):
        bco[b + 1] = bco[b] + slots_bq[b].sum() // CHUNK

    plan = dict(
        N=N, E=E, n_cores=n_cores, npc=npc, nsw=nsw, npc_pad=npc_pad,
        swb=swb, nb=nb, qr=qr, cwq=cwq, slots_bq=slots_bq, tot=tot,
        totch=totch, gbase=gbase, off16=off16, offch=offch, bco=bco,
    )

    # ---- pack per-core index/dstloc arrays ----
    order = np.lexsort((q, sw, core))
    so_src, so_dl, so_sw, so_q, so_core = (
        src[order], dl[order], sw[order], q[order], core[order]
    )
    core_starts = np.searchsorted(so_core, np.arange(n_cores + 1))

    per_core = []
    for c in range(n_cores):
        lo, hi = core_starts[c], core_starts[c + 1]
        cs, cdl, csw, cq = so_src[lo:hi], so_dl[lo:hi], so_sw[lo:hi], so_q[lo:hi]
        # slot arrays for gather idx (int16, pad=0) and dstloc (f32, pad=-1)
        gvals = np.zeros(tot, np.int16)
        dvals = np.full(tot, 1.0, np.float32)
        # group boundaries within this core's (sw-major, q-minor) sorted edges
        ggid = csw * NQ + cq
        starts = np.searchsorted(ggid, np.arange(nsw * NQ + 1))
        for s in range(nsw):
            b = s // swb
            for qq in range(NQ):
                g0, g1 = starts[s * NQ + qq], starts[s * NQ + qq + 1]
                n = g1 - g0
                if n == 0:
                    continue
                base = (off16[b, qq] * 16
                        + (gbase[s, qq] + 0) * CHUNK)
                gvals[base:base + n] = (cs[g0:g1] - cq[g0:g1] * qr).astype(np.int16)
                # stored NEGATED: DVE compares -iota == -dl, ACT computes
                # |iota + (-dl)|; pads hold +1 (never matches either way)
                dvals[base:base + n] = -(cdl[g0:g1] - s * SWW).astype(np.float32)
        # wrap into the idxs_ap layout: [(b,q) segment] -> [16, seg/16],
        # replicated down 128 partitions
        gidx = np.empty((128, tot // 16), np.int16)
        dstc = np.empty((128, totch), np.float32)
        for b in range(nb):
            for qq in range(NQ):
                sl = int(slots_bq[b, qq])
                seg = gvals[off16[b, qq] * 16: off16[b, qq] * 16 + sl]
                w = seg.reshape(-1, 16).T                      # [16, sl/16]
                gidx[:, off16[b, qq]: off16[b, qq] + sl // 16] = np.tile(w, (8, 1))
                dseg = dvals[off16[b, qq] * 16: off16[b, qq] * 16 + sl]
                dstc[:, offch[b, qq]: offch[b, qq] + sl // CHUNK] = (
                    dseg.reshape(-1, CHUNK).T
                )
        xt = np.zeros((D, npc_pad), np.float32)
        n_real = min(npc, N - c * npc)
        xt[:, :n_real] = np.asarray(x[c * npc: c * npc + n_real]).T
        per_core.append(dict(gidx=gidx, dstc=dstc, xt=xt))

    return plan, per_core


def _build_nc(plan):
    import concourse.bacc as bacc
    import concourse.mybir as mybir
    import concourse.tile as tile

    f16 = mybir.dt.float16
    bf16 = mybir.dt.bfloat16
    f32 = mybir.dt.float32
    i16 = mybir.dt.int16

    N, nb, swb, nsw = plan["N"], plan["nb"], plan["swb"], plan["nsw"]
    qr, npc_pad = plan["qr"], plan["npc_pad"]
    cwq, slots_bq = plan["cwq"], plan["slots_bq"]
    off16, offch, gbase, bco = plan["off16"], plan["offch"], plan["gbase"], plan["bco"]
    n_pad_rows = qr * NQ                      # xpad row count (>= N)

    # 4 SWDGE queues: each dma_gather runs on the Q7 core pair selected by
    # queue_num (ucode: cpu_id/2 == queue_num), so spreading the 4 quartile
    # gathers across queues 0-3 engages all 8 gpsimd cores in parallel
    # (measured 2.8x on descriptor generation, the kernel bottleneck).
    nc = bacc.Bacc("TRN2", num_swdge_queues=4)
    xpad_d = nc.dram_tensor("xpad", [n_pad_rows, DP], f16, kind="ExternalInput")
    gidx_d = nc.dram_tensor("gidx", [128, plan["tot"] // 16], i16, kind="ExternalInput")
    dstc_d = nc.dram_tensor("dstc", [128, plan["totch"]], f32, kind="ExternalInput")
    xt_d = nc.dram_tensor("xt", [D, npc_pad], f32, kind="ExternalInput")
    iota_d = nc.dram_tensor("iota", [128, SWW], bf16, kind="ExternalInput")
    iotan_d = nc.dram_tensor("iotan", [128, SWW], bf16, kind="ExternalInput")
    w1_d = nc.dram_tensor("w1", [D, D], f16, kind="ExternalInput")
    w2_d = nc.dram_tensor("w2", [D, D], f16, kind="ExternalInput")
    b1_d = nc.dram_tensor("b1", [D, 1], f32, kind="ExternalInput")
    b2_d = nc.dram_tensor("b2", [D, 1], f32, kind="ExternalInput")
    out_d = nc.dram_tensor("outT", [D, npc_pad], f32, kind="ExternalOutput")

    bw = swb * SWW                            # dst cols per batch

    with tile.TileContext(nc) as tc:
        with (
            tc.tile_pool(name="const", bufs=1) as cpool,
            tc.tile_pool(name="idx", bufs=2) as ipool,
            tc.tile_pool(name="g", bufs=2) as gpool,
            tc.tile_pool(name="meta", bufs=2) as mpool,
            tc.tile_pool(name="oh", bufs=8) as ohpool,
            tc.tile_pool(name="act", bufs=4) as apool,
            tc.tile_pool(name="ost", bufs=2) as opool,
            tc.tile_pool(name="psA", bufs=3, space="PSUM") as psA,
            tc.tile_pool(name="psB", bufs=2, space="PSUM") as psB,
        ):
            iota_t = cpool.tile([128, SWW], bf16, tag="iota")
            nc.sync.dma_start(iota_t[:], iota_d[:])
            iotan_t = cpool.tile([128, SWW], bf16, tag="iotan")
            nc.sync.dma_start(iotan_t[:], iotan_d[:])
            w1_t = cpool.tile([D, D], f16, tag="w1")
            nc.sync.dma_start(w1_t[:], w1_d[:])
            w2_t = cpool.tile([D, D], f16, tag="w2")
            nc.sync.dma_start(w2_t[:], w2_d[:])
            b1_t = cpool.tile([D, 1], f32, tag="b1")
            nc.sync.dma_start(b1_t[:], b1_d[:])
            b2_t = cpool.tile([D, 1], f32, tag="b2")
            nc.sync.dma_start(b2_t[:], b2_d[:])

            for b in range(nb):
                nch_b = int(bco[b + 1] - bco[b])
                dst_t = mpool.tile([128, nch_b], f32, tag="dst")
                nc.sync.dma_start(dst_t[:], dstc_d[:, int(bco[b]):int(bco[b + 1])])
                xt_t = mpool.tile([D, bw], f32, tag="xt")
                nc.sync.dma_start(xt_t[:], xt_d[:, b * bw:(b + 1) * bw])

                g_ap = {}
                for qq in range(NQ):
                    sl = int(slots_bq[b, qq])
                    it = ipool.tile([128, sl // 16], i16, tag=f"i{qq}")
                    nc.sync.dma_start(
                        it[:], gidx_d[:, int(off16[b, qq]): int(off16[b, qq]) + sl // 16]
                    )
                    gt = gpool.tile([128, (sl // CHUNK) * DP], f16, tag=f"g{qq}")
                    ga = gt[:].rearrange("p (c e) -> p c e", e=DP)
                    # SWDGE ring fits <16384 descriptors per call; split
                    # defensively at 8192 (slices stay 16/128-aligned).
                    for s0 in range(0, sl, 8192):
                        s1 = min(s0 + 8192, sl)
                        nc.gpsimd.dma_gather(
                            ga[:, s0 // CHUNK: s1 // CHUNK, :],
                            xpad_d[qq * qr:(qq + 1) * qr, :],
                            it[:, s0 // 16: s1 // 16],
                            s1 - s0, s1 - s0, DP,
                            single_packet=False, queue_num=qq,
                        )
                    g_ap[qq] = ga

                ost = opool.tile([D, bw], f32, tag="ost")
                for j in range(swb):
                    s = b * swb + j
                    agg = psA.tile([D, SWW], f32, tag="agg")
                    nmm = int(cwq[s].sum())
                    i = 0
                    for qq in range(NQ):
                        for k in range(int(cwq[s, qq])):
                            col = int(offch[b, qq] - bco[b] + gbase[s, qq] + k)
                            slot = int(gbase[s, qq] + k)
                            # bf16 one-hot: PE accepts the fp16(lhsT) x
                            # bf16(rhs) mix.  ~30% of the builds go to the
                            # otherwise-idle scalar engine as
                            # Relu(1 - |iota - dl|), the rest run as
                            # is_equal(-iota, -dl) on DVE.
                            oh = ohpool.tile([128, SWW], bf16, tag="oh")
                            if i % 10 < 3:
                                u = ohpool.tile([128, SWW], bf16, tag="ohu")
                                nc.scalar.activation(
                                    u[:], iota_t[:],
                                    mybir.ActivationFunctionType.Abs,
                                    bias=dst_t[:, col:col + 1],
                                )
                                nc.scalar.activation(
                                    oh[:], u[:],
                                    mybir.ActivationFunctionType.Relu,
                                    bias=1.0, scale=-1.0,
                                )
                            else:
                                nc.vector.tensor_scalar(
                                    oh[:], iotan_t[:], dst_t[:, col:col + 1],
                                    None, mybir.AluOpType.is_equal,
                                )
                            nc.tensor.matmul(
                                agg[:], g_ap[qq][:, slot, 0:D], oh[:],
                                start=(i == 0), stop=(i == nmm - 1),
                            )
                            i += 1
                    hT = apool.tile([D, SWW], f16, tag="hT")
                    nc.vector.tensor_add(hT[:], agg[:], xt_t[:, j * SWW:(j + 1) * SWW])
                    z1 = psB.tile([D, SWW], f32, tag="z1")
                    nc.tensor.matmul(z1[:], w1_t[:], hT[:])
                    a1 = apool.tile([D, SWW], f16, tag="a1")
                    nc.scalar.activation(
                        a1[:], z1[:], mybir.ActivationFunctionType.Relu,
                        bias=b1_t[:, 0:1],
                    )
                    z2 = psB.tile([D, SWW], f32, tag="z2")
                    nc.tensor.matmul(z2[:], w2_t[:], a1[:])
                    nc.scalar.activation(
                        ost[:, j * SWW:(j + 1) * SWW], z2[:],
                        mybir.ActivationFunctionType.Identity, bias=b2_t[:, 0:1],
                    )
                nc.sync.dma_start(out_d[:, b * bw:(b + 1) * bw], ost[:])
    return nc


def _shared_inputs(x, W1, b1, W2, b2, plan):
    N = plan["N"]
    qr = plan["qr"]
    xpad = np.zeros((qr * NQ, DP), np.float16)
    xpad[:N, :D] = np.asarray(x, np.float32).astype(np.float16)
    import ml_dtypes
    return dict(
        xpad=xpad,
        iota=np.broadcast_to(
            np.arange(SWW, dtype=ml_dtypes.bfloat16), (128, SWW)
        ).copy(),
        iotan=np.broadcast_to(
            -np.arange(SWW, dtype=ml_dtypes.bfloat16), (128, SWW)
        ).copy(),
        w1=np.asarray(W1, np.float32).astype(np.float16),
        w2=np.asarray(W2, np.float32).astype(np.float16),
        b1=np.asarray(b1, np.float32).reshape(D, 1),
        b2=np.asarray(b2, np.float32).reshape(D, 1),
    )


def kernel(x, edge_index, W1, b1, W2, b2):
    from concourse.bass_utils import run_bass_kernel_spmd

    x = np.asarray(x)
    n_cores = 8
    plan, per_core = _plan_and_pack(x, edge_index, n_cores)
    shared = _shared_inputs(x, W1, b1, W2, b2, plan)
    in_maps = [{**shared, **pc} for pc in per_core]

    nc = _build_nc(plan)
    nc.finalize()
    res = run_bass_kernel_spmd(nc, in_maps, core_ids=list(range(n_cores)))

    N, npc = plan["N"], plan["npc"]
    out = np.empty((N, D), np.float32)
    for c in range(n_cores):
        n_real = min(npc, N - c * npc)
        out[c * npc: c * npc + n_real] = res.results[c]["outT"][:, :n_real].T
    return out


# revision 17
# speedup vs baseline: 1.5445x; 1.5445x over previous
"""GINConv (sum-aggregation + 2-layer MLP) on 8 Trainium2 NeuronCores.

Strategy (see sharding hint): shard destination nodes across the 8 cores
(12500 rows each).  Edges are partitioned by destination core.  Inside a
core, edges are grouped by 256-wide destination "super-windows" and by
source quartile (so gather indices fit in int16).  The per-edge source
features are fetched with the HW dma_gather (SWDGE) primitive from a
replicated fp16 copy of x; the scatter-add is performed locally on the
tensor engine as  aggT[64f x 256d] += G_chunk[128e x 64f]^T @ onehot[128e x 256d]
with the one-hot built on the vector engine via tensor_scalar(is_equal)
against an iota row.  The MLP runs in the same transposed orientation
(features on partitions), so no transposes are needed on device; the host
transposes the final [64, 12544] output tiles back.

All 8 cores execute one identical NEFF (SPMD) - the chunk grid is padded
to the max across cores so the static schedule is core-independent.
"""

import numpy as np

D = 64          # feature dim
DP = 128        # padded feature dim (fp16 row = 256B, dma_gather elem size)
SWW = 256       # dsts per super-window (psum tile free dim)
NQ = 4          # source quartiles (gather idx must fit int16)
CHUNK = 128     # edges per matmul chunk (PE contraction dim)


def _plan_and_pack(x, edge_index, n_cores=8, swb=None):
    """Host-side: shard/sort/pad edges; build per-core packed index arrays.

    Returns (plan, per_core_inputs).
    """
    N = x.shape[0]
    E = edge_index.shape[1]
    npc = -(-N // n_cores)                  # nodes per core
    nsw = -(-npc // SWW)                    # super-windows per core
    npc_pad = nsw * SWW
    if swb is None:
        swb = 7 if nsw % 7 == 0 else 1      # super-windows per batch
    while nsw % swb != 0:
        swb -= 1
    nb = nsw // swb                         # batches
    qr = -(-N // NQ)                        # rows per source quartile
    assert qr <= 32767

    src = np.asarray(edge_index[0], dtype=np.int64)
    dst = np.asarray(edge_index[1], dtype=np.int64)

    # group sizes per (core, sw, q)
    core = dst // npc
    dl = dst - core * npc
    sw = dl // SWW
    q = src // qr
    gid = (core * nsw + sw) * NQ + q
    counts = np.bincount(gid, minlength=n_cores * nsw * NQ).reshape(
        n_cores, nsw, NQ
    )
    # uniform chunk grid: max over cores, at least 1 chunk
    cwq = np.maximum(1, -(-counts.max(axis=0) // CHUNK))      # [nsw, NQ]
    slots_g = cwq * CHUNK                                     # [nsw, NQ]

    # per (batch, q) stream sizes and offsets
    slots_bq = np.zeros((nb, NQ), np.int64)
    for b in range(nb):
        slots_bq[b] = slots_g[b * swb:(b + 1) * swb].sum(axis=0)
    tot = int(slots_bq.sum())
    totch = tot // CHUNK

    # chunk-base of group (sw, q) inside its (b, q) stream
    gbase = np.zeros((nsw, NQ), np.int64)
    for b in range(nb):
        acc = np.zeros(NQ, np.int64)
        for s in range(b * swb, (b + 1) * swb):
            gbase[s] = acc
            acc += cwq[s]
    # free-dim int16-col offset of (b, q) idx stream in the packed gidx array
    off16 = np.zeros((nb, NQ), np.int64)
    # chunk-column offset of (b, q) inside the packed dstloc array
    offch = np.zeros((nb, NQ), np.int64)
    a16 = 0
    ach = 0
    for b in range(nb):
        for qq in range(NQ):
            off16[b, qq] = a16
            offch[b, qq] = ach
            a16 += slots_bq[b, qq] // 16
            ach += slots_bq[b, qq] // CHUNK
    bco = np.zeros(nb + 1, np.int64)        # batch chunk-col starts
    for b in range(nb):
        bco[b + 1] = bco[b] + slots_bq[b].sum() // CHUNK

    plan = dict(
        N=N, E=E, n_cores=n_cores, npc=npc, nsw=nsw, npc_pad=npc_pad,
        swb=swb, nb=nb, qr=qr, cwq=cwq, slots_bq=slots_bq, tot=tot,
        totch=totch, gbase=gbase, off16=off16, offch=offch, bco=bco,
    )

    # ---- pack per-core index/dstloc arrays ----
    order = np.lexsort((q, sw, core))
    so_src, so_dl, so_sw, so_q, so_core = (
        src[order], dl[order], sw[order], q[order], core[order]
    )
    core_starts = np.searchsorted(so_core, np.arange(n_cores + 1))

    per_core = []
    for c in range(n_cores):
        lo, hi = core_starts[c], core_starts[c + 1]
        cs, cdl, csw, cq = so_src[lo:hi], so_dl[lo:hi], so_sw[lo:hi], so_q[lo:hi]
        # slot arrays for gather idx (int16, pad=0) and dstloc (f32, pad=-1)
        gvals = np.zeros(tot, np.int16)
        dvals = np.full(tot, -1.0, np.float32)
        # group boundaries within this core's (sw-major, q-minor) sorted edges
        ggid = csw * NQ + cq
        starts = np.searchsorted(ggid, np.arange(nsw * NQ + 1))
        for s in range(nsw):
            b = s // swb
            for qq in range(NQ):
                g0, g1 = starts[s * NQ + qq], starts[s * NQ + qq + 1]
                n = g1 - g0
                if n == 0:
                    continue
                base = (off16[b, qq] * 16
                        + (gbase[s, qq] + 0) * CHUNK)
                gvals[base:base + n] = (cs[g0:g1] - cq[g0:g1] * qr).astype(np.int16)
                dvals[base:base + n] = (cdl[g0:g1] - s * SWW).astype(np.float32)
        # wrap into the idxs_ap layout: [(b,q) segment] -> [16, seg/16],
        # replicated down 128 partitions
        gidx = np.empty((128, tot // 16), np.int16)
        dstc = np.empty((128, totch), np.float32)
        for b in range(nb):
            for qq in range(NQ):
                sl = int(slots_bq[b, qq])
                seg = gvals[off16[b, qq] * 16: off16[b, qq] * 16 + sl]
                w = seg.reshape(-1, 16).T                      # [16, sl/16]
                gidx[:, off16[b, qq]: off16[b, qq] + sl // 16] = np.tile(w, (8, 1))
                dseg = dvals[off16[b, qq] * 16: off16[b, qq] * 16 + sl]
                dstc[:, offch[b, qq]: offch[b, qq] + sl // CHUNK] = (
                    dseg.reshape(-1, CHUNK).T
                )
        xt = np.zeros((D, npc_pad), np.float32)
        n_real = min(npc, N - c * npc)
        xt[:, :n_real] = np.asarray(x[c * npc: c * npc + n_real]).T
        per_core.append(dict(gidx=gidx, dstc=dstc, xt=xt))

    return plan, per_core


def _build_nc(plan):
    import concourse.bacc as bacc
    import concourse.mybir as mybir
    import concourse.tile as tile

    f16 = mybir.dt.float16
    bf16 = mybir.dt.bfloat16
    f32 = mybir.dt.float32
    i16 = mybir.dt.int16

    N, nb, swb, nsw = plan["N"], plan["nb"], plan["swb"], plan["nsw"]
    qr, npc_pad = plan["qr"], plan["npc_pad"]
    cwq, slots_bq = plan["cwq"], plan["slots_bq"]
    off16, offch, gbase, bco = plan["off16"], plan["offch"], plan["gbase"], plan["bco"]
    n_pad_rows = qr * NQ                      # xpad row count (>= N)

    # 4 SWDGE queues: each dma_gather runs on the Q7 core pair selected by
    # queue_num (ucode: cpu_id/2 == queue_num), so spreading the 4 quartile
    # gathers across queues 0-3 engages all 8 gpsimd cores in parallel
    # (measured 2.8x on descriptor generation, the kernel bottleneck).
    nc = bacc.Bacc("TRN2", num_swdge_queues=4)
    xpad_d = nc.dram_tensor("xpad", [n_pad_rows, DP], f16, kind="ExternalInput")
    gidx_d = nc.dram_tensor("gidx", [128, plan["tot"] // 16], i16, kind="ExternalInput")
    dstc_d = nc.dram_tensor("dstc", [128, plan["totch"]], f32, kind="ExternalInput")
    xt_d = nc.dram_tensor("xt", [D, npc_pad], f32, kind="ExternalInput")
    iota_d = nc.dram_tensor("iota", [128, SWW], bf16, kind="ExternalInput")
    w1_d = nc.dram_tensor("w1", [D, D], f16, kind="ExternalInput")
    w2_d = nc.dram_tensor("w2", [D, D], f16, kind="ExternalInput")
    b1_d = nc.dram_tensor("b1", [D, 1], f32, kind="ExternalInput")
    b2_d = nc.dram_tensor("b2", [D, 1], f32, kind="ExternalInput")
    out_d = nc.dram_tensor("outT", [D, npc_pad], f32, kind="ExternalOutput")

    bw = swb * SWW                            # dst cols per batch

    with tile.TileContext(nc) as tc:
        with (
            tc.tile_pool(name="const", bufs=1) as cpool,
            tc.tile_pool(name="idx", bufs=2) as ipool,
            tc.tile_pool(name="g", bufs=2) as gpool,
            tc.tile_pool(name="meta", bufs=2) as mpool,
            tc.tile_pool(name="oh", bufs=8) as ohpool,
            tc.tile_pool(name="act", bufs=4) as apool,
            tc.tile_pool(name="ost", bufs=2) as opool,
            tc.tile_pool(name="psA", bufs=3, space="PSUM") as psA,
            tc.tile_pool(name="psB", bufs=2, space="PSUM") as psB,
        ):
            iota_t = cpool.tile([128, SWW], bf16, tag="iota")
            nc.sync.dma_start(iota_t[:], iota_d[:])
            w1_t = cpool.tile([D, D], f16, tag="w1")
            nc.sync.dma_start(w1_t[:], w1_d[:])
            w2_t = cpool.tile([D, D], f16, tag="w2")
            nc.sync.dma_start(w2_t[:], w2_d[:])
            b1_t = cpool.tile([D, 1], f32, tag="b1")
            nc.sync.dma_start(b1_t[:], b1_d[:])
            b2_t = cpool.tile([D, 1], f32, tag="b2")
            nc.sync.dma_start(b2_t[:], b2_d[:])

            for b in range(nb):
                nch_b = int(bco[b + 1] - bco[b])
                dst_t = mpool.tile([128, nch_b], f32, tag="dst")
                nc.sync.dma_start(dst_t[:], dstc_d[:, int(bco[b]):int(bco[b + 1])])
                xt_t = mpool.tile([D, bw], f32, tag="xt")
                nc.sync.dma_start(xt_t[:], xt_d[:, b * bw:(b + 1) * bw])

                g_ap = {}
                for qq in range(NQ):
                    sl = int(slots_bq[b, qq])
                    it = ipool.tile([128, sl // 16], i16, tag=f"i{qq}")
                    nc.sync.dma_start(
                        it[:], gidx_d[:, int(off16[b, qq]): int(off16[b, qq]) + sl // 16]
                    )
                    gt = gpool.tile([128, (sl // CHUNK) * DP], f16, tag=f"g{qq}")
                    ga = gt[:].rearrange("p (c e) -> p c e", e=DP)
                    # SWDGE ring fits <16384 descriptors per call; split
                    # defensively at 8192 (slices stay 16/128-aligned).
                    for s0 in range(0, sl, 8192):
                        s1 = min(s0 + 8192, sl)
                        nc.gpsimd.dma_gather(
                            ga[:, s0 // CHUNK: s1 // CHUNK, :],
                            xpad_d[qq * qr:(qq + 1) * qr, :],
                            it[:, s0 // 16: s1 // 16],
                            s1 - s0, s1 - s0, DP,
                            single_packet=False, queue_num=qq,
                        )
                    g_ap[qq] = ga

                ost = opool.tile([D, bw], f32, tag="ost")
                for j in range(swb):
                    s = b * swb + j
                    agg = psA.tile([D, SWW], f32, tag="agg")
                    nmm = int(cwq[s].sum())
                    i = 0
                    for qq in range(NQ):
                        for k in range(int(cwq[s, qq])):
                            col = int(offch[b, qq] - bco[b] + gbase[s, qq] + k)
                            slot = int(gbase[s, qq] + k)
                            # bf16 one-hot: DVE runs 16-bit bf16 tensor_scalar
                            # in a fast mode fp16 doesn't get; PE accepts the
                            # fp16(lhsT) x bf16(rhs) mix.
                            oh = ohpool.tile([128, SWW], bf16, tag="oh")
                            nc.vector.tensor_scalar(
                                oh[:], iota_t[:], dst_t[:, col:col + 1], None,
                                mybir.AluOpType.is_equal,
                            )
                            nc.tensor.matmul(
                                agg[:], g_ap[qq][:, slot, 0:D], oh[:],
                                start=(i == 0), stop=(i == nmm - 1),
                            )
                            i += 1
                    hT = apool.tile([D, SWW], f16, tag="hT")
                    nc.vector.tensor_add(hT[:], agg[:], xt_t[:, j * SWW:(j + 1) * SWW])
                    z1 = psB.tile([D, SWW], f32, tag="z1")
                    nc.tensor.matmul(z1[:], w1_t[:], hT[:])
                    a1 = apool.tile([D, SWW], f16, tag="a1")
                    nc.scalar.activation(
                        a1[:], z1[:], mybir.ActivationFunctionType.Relu,
                        bias=b1_t[:, 0:1],
                    )
                    z2 = psB.tile([D, SWW], f32, tag="z2")
                    nc.tensor.matmul(z2[:], w2_t[:], a1[:])
                    nc.scalar.activation(
                        ost[:, j * SWW:(j + 1) * SWW], z2[:],
                        mybir.ActivationFunctionType.Identity, bias=b2_t[:, 0:1],
                    )
                nc.sync.dma_start(out_d[:, b * bw:(b + 1) * bw], ost[:])
    return nc


def _shared_inputs(x, W1, b1, W2, b2, plan):
    N = plan["N"]
    qr = plan["qr"]
    xpad = np.zeros((qr * NQ, DP), np.float16)
    xpad[:N, :D] = np.asarray(x, np.float32).astype(np.float16)
    import ml_dtypes
    return dict(
        xpad=xpad,
        iota=np.broadcast_to(
            np.arange(SWW, dtype=ml_dtypes.bfloat16), (128, SWW)
        ).copy(),
        w1=np.asarray(W1, np.float32).astype(np.float16),
        w2=np.asarray(W2, np.float32).astype(np.float16),
        b1=np.asarray(b1, np.float32).reshape(D, 1),
        b2=np.asarray(b2, np.float32).reshape(D, 1),
    )


def kernel(x, edge_index, W1, b1, W2, b2):
    from concourse.bass_utils import run_bass_kernel_spmd

    x = np.asarray(x)
    n_cores = 8
    plan, per_core = _plan_and_pack(x, edge_index, n_cores)
    shared = _shared_inputs(x, W1, b1, W2, b2, plan)
    in_maps = [{**shared, **pc} for pc in per_core]

    nc = _build_nc(plan)
    nc.finalize()
    res = run_bass_kernel_spmd(nc, in_maps, core_ids=list(range(n_cores)))

    N, npc = plan["N"], plan["npc"]
    out = np.empty((N, D), np.float32)
    for c in range(n_cores):
        n_real = min(npc, N - c * npc)
        out[c * npc: c * npc + n_real] = res.results[c]["outT"][:, :n_real].T
    return out
